# revision 3
# baseline (speedup 1.0000x reference)
"""Trainium2 Bass kernel for DiffMultiHeadedAttention (differential attention).

Model (per reference):
    q = x @ Wq.T + bq                 [B,T,1024]
    k = ef @ Wk.T + bk                [B,N,1024]
    v = ef @ Wv.T + bv                [B,N,1024]
    lambda_full = exp(sum(lq1*lk1)) - exp(sum(lq2*lk2)) + 0.8
    att  = softmax(causal_mask(q_hh @ k_hh.T / sqrt(32)))   per 32 half-heads
    out_h = att[2h] - lambda_full * att[2h+1]  @ v_h        per 16 heads
B=4, T=N=1024, H=16 heads of 64, 2H=32 half-heads of 32.

Sharding over 8 cores: core c = (batch b = c//2, head-group hg = c%2).
Each core owns one batch element and 8 full heads (16 half-heads) and
computes out^T [512, 1024] for its (b, head-slice); the host transposes
and reassembles the full [4, 1024, 1024] output.

On-core dataflow (all fp32):
  - PE transposes x[b], ef[b] and the 512-row weight slices (fp32 has no
    DMA transpose) to get contraction-major layouts.
  - qT = (WqT).T @ xT per o-chunk, kT likewise from efT, v = efT.T @ WvT.
  - Attention in transposed layout: attT[n,t] = kT.T @ qT (K=32 row-tiled
    pairs), E = exp(attT/sqrt(32)) with triangular mask on the diagonal
    128x128 block, causally skipping fully-masked n-tiles.
  - av: outT_psum[65, t] += [v_h | ones].T @ E  accumulated over n-chunks;
    row 64 gives the softmax denominators for free.
  - Combine: out = P_pos/s_pos - lambda * P_neg/s_neg via batched DVE
    reciprocal + gpsimd partition-broadcast, DMA'd out as outT [512,1024].
"""

import math

import numpy as np

B, T, N, HIDDEN = 4, 1024, 1024, 1024
H, HEAD, HALF = 16, 64, 32
O = 512            # per-core hidden slice (8 heads * 64)
HPC = 8            # heads per core
LAMBDA_INIT = 0.8
SCALE = 1.0 / math.sqrt(HALF)
P = 128
IC = HIDDEN // P   # 8 contraction chunks
OC = O // P        # 4 output chunks of the projections
NT = N // P        # 8 n-tiles (keys)
NCORES = 8

_STATE = {}


def _build_nc():
    from contextlib import ExitStack

    import concourse.bacc as bacc
    import concourse.mybir as mybir
    import concourse.tile as tile
    from concourse.bass import ts
    from concourse.masks import make_identity

    f32 = mybir.dt.float32
    AF = mybir.ActivationFunctionType
    ALU = mybir.AluOpType

    nc = bacc.Bacc("TRN2", target_bir_lowering=False, debug=False)

    x_d = nc.dram_tensor("x", [T, HIDDEN], f32, kind="ExternalInput")
    ef_d = nc.dram_tensor("ef", [N, HIDDEN], f32, kind="ExternalInput")
    wq_d = nc.dram_tensor("wq", [O, HIDDEN], f32, kind="ExternalInput")
    wk_d = nc.dram_tensor("wk", [O, HIDDEN], f32, kind="ExternalInput")
    wv_d = nc.dram_tensor("wv", [O, HIDDEN], f32, kind="ExternalInput")
    bq_d = nc.dram_tensor("bq", [1, O], f32, kind="ExternalInput")
    bk_d = nc.dram_tensor("bk", [1, O], f32, kind="ExternalInput")
    bv_d = nc.dram_tensor("bv", [1, O], f32, kind="ExternalInput")
    lq1_d = nc.dram_tensor("lq1", [1, HALF], f32, kind="ExternalInput")
    lq2_d = nc.dram_tensor("lq2", [1, HALF], f32, kind="ExternalInput")
    lk1_d = nc.dram_tensor("lk1", [1, HALF], f32, kind="ExternalInput")
    lk2_d = nc.dram_tensor("lk2", [1, HALF], f32, kind="ExternalInput")
    outT_d = nc.dram_tensor("outT", [O, T], f32, kind="ExternalOutput")

    with tile.TileContext(nc) as tc:
        with ExitStack() as ctx:
            const = ctx.enter_context(tc.tile_pool(name="const", bufs=1))
            ident = const.tile([P, P], f32)
            make_identity(nc, ident)

            # ---- lambda_full (tiny, computed once) ----
            lam_in = const.tile([1, 4, HALF], f32)
            nc.sync.dma_start(lam_in[:, 0, :], lq1_d[:])
            nc.sync.dma_start(lam_in[:, 1, :], lk1_d[:])
            nc.sync.dma_start(lam_in[:, 2, :], lq2_d[:])
            nc.sync.dma_start(lam_in[:, 3, :], lk2_d[:])
            lam_tmp = const.tile([1, 2, HALF], f32)
            nc.vector.tensor_mul(lam_tmp[:, 0, :], lam_in[:, 0, :], lam_in[:, 1, :])
            nc.vector.tensor_mul(lam_tmp[:, 1, :], lam_in[:, 2, :], lam_in[:, 3, :])
            lam_s = const.tile([1, 2], f32)
            nc.vector.tensor_reduce(
                lam_s, lam_tmp, axis=mybir.AxisListType.X, op=ALU.add
            )
            lam_e = const.tile([1, 2], f32)
            nc.scalar.activation(lam_e, lam_s, AF.Exp)
            # lam_neg = -(e1 - e2 + 0.8) = e2 - e1 - 0.8
            lam_neg = const.tile([1, 1], f32)
            nc.vector.tensor_sub(lam_neg, lam_e[:, 1:2], lam_e[:, 0:1])
            nc.vector.tensor_scalar_add(lam_neg, lam_neg, -LAMBDA_INIT)
            lam_neg_b = const.tile([64, 1], f32)
            nc.gpsimd.partition_broadcast(lam_neg_b, lam_neg)

            # ---- biases ----
            bq_sb = const.tile([P, OC], f32)
            nc.sync.dma_start(bq_sb, bq_d[0].rearrange("(a p) -> p a", p=P))
            bk_sb = const.tile([P, OC], f32)
            nc.sync.dma_start(bk_sb, bk_d[0].rearrange("(a p) -> p a", p=P))
            bv_1 = const.tile([1, O], f32)
            nc.sync.dma_start(bv_1, bv_d[:])
            bvb = const.tile([P, O], f32)
            nc.gpsimd.partition_broadcast(bvb, bv_1)

            # ---- persistent projection outputs ----
            proj = ctx.enter_context(tc.tile_pool(name="proj", bufs=1))
            qT = proj.tile([P, OC, T], f32)          # [d-part, oc, t]
            kT = proj.tile([P, OC, N], f32)          # [d-part, oc, n]
            vaug = proj.tile([P, NT, HPC, HEAD + 1], f32)  # [n-part, nt, h, d|1]
            for nt_ in range(NT):
                nc.vector.memset(vaug[:, nt_, :, HEAD : HEAD + 1], 1.0)

            # =============== phase 1: transposes + projections ===============
            with (
                tc.tile_pool(name="ph1", bufs=2) as ph1,
                tc.tile_pool(name="big", bufs=1) as big,
                tc.tile_pool(name="ps_tr", bufs=4, space="PSUM") as ps_tr,
                tc.tile_pool(name="ps_pj", bufs=4, space="PSUM") as ps_pj,
            ):
                xT = big.tile([P, IC, T], f32)
                efT = big.tile([P, IC, N], f32)
                for src_d, dstT in ((x_d, xT), (ef_d, efT)):
                    for tt in range(T // P):
                        nat = ph1.tile([P, HIDDEN], f32, tag="nat", name=f"nat{tt}")
                        nc.sync.dma_start(nat, src_d[ts(tt, P), :])
                        for ic in range(IC):
                            pst = ps_tr.tile([P, P], f32, tag="tr", name="pst")
                            nc.tensor.transpose(pst, nat[:, ts(ic, P)], ident)
                            nc.vector.tensor_copy(dstT[:, ic, ts(tt, P)], pst)

                # q/k projections: qT[o,t] = sum_ic WqT[ic].T @ xT[ic]  (+bias)
                for w_d, b_sb, actT, dstT in (
                    (wq_d, bq_sb, xT, qT),
                    (wk_d, bk_sb, efT, kT),
                ):
                    for oc in range(OC):
                        wnat = ph1.tile([P, HIDDEN], f32, tag="nat", name="wnat")
                        nc.sync.dma_start(wnat, w_d[ts(oc, P), :])
                        wT = ph1.tile([P, IC, P], f32, tag="wT", name="wT")
                        for ic in range(IC):
                            pst = ps_tr.tile([P, P], f32, tag="tr", name="pst")
                            nc.tensor.transpose(pst, wnat[:, ts(ic, P)], ident)
                            nc.vector.tensor_copy(wT[:, ic, :], pst)
                        for t2 in range(2):
                            psj = ps_pj.tile([P, 512], f32, tag="pj", name="psj")
                            for ic in range(IC):
                                nc.tensor.matmul(
                                    psj,
                                    wT[:, ic, :],
                                    actT[:, ic, ts(t2, 512)],
                                    start=(ic == 0),
                                    stop=(ic == IC - 1),
                                )
                            nc.vector.tensor_scalar_add(
                                dstT[:, oc, ts(t2, 512)], psj, b_sb[:, oc : oc + 1]
                            )

                # v projection: v[n, o] = sum_ic efT[ic].T @ WvT[ic]  (+bias)
                wvT = big.tile([P, IC, O], f32)
                for oc in range(OC):
                    wnat = ph1.tile([P, HIDDEN], f32, tag="nat", name="wvnat")
                    nc.sync.dma_start(wnat, wv_d[ts(oc, P), :])
                    for ic in range(IC):
                        pst = ps_tr.tile([P, P], f32, tag="tr", name="pst")
                        nc.tensor.transpose(pst, wnat[:, ts(ic, P)], ident)
                        nc.vector.tensor_copy(wvT[:, ic, ts(oc, P)], pst)
                for nt_ in range(NT):
                    psj = ps_pj.tile([P, 512], f32, tag="pj", name="psv")
                    for ic in range(IC):
                        nc.tensor.matmul(
                            psj,
                            efT[:, ic, ts(nt_, P)],
                            wvT[:, ic, :],
                            start=(ic == 0),
                            stop=(ic == IC - 1),
                        )
                    nc.vector.tensor_add(
                        vaug[:, nt_, :, 0:HEAD],
                        psj[:].rearrange("p (h d) -> p h d", h=HPC),
                        bvb[:].rearrange("p (h d) -> p h d", h=HPC),
                    )

            # =============== phase 2: attention ===============
            with (
                tc.tile_pool(name="att_sb", bufs=4) as att_sb,
                tc.tile_pool(name="acc_sb", bufs=1) as acc_sb,
                tc.tile_pool(name="ps_qk", bufs=2, space="PSUM") as ps_qk,
                tc.tile_pool(name="ps_av", bufs=2, space="PSUM") as ps_av,
            ):
                # P65[:, h, s, t]: rows 0..63 = (E_s @ v_h).T, row 64 = sum_n E_s
                P65 = acc_sb.tile([65, HPC, 2, T], f32)
                S_sb = acc_sb.tile([16, T], f32)

                for oc in range(OC):
                    for j in range(2):
                        h = 2 * oc + j
                        av_ps = [
                            ps_av.tile([65, 2, 512], f32, tag="av", name=f"av{h}_{t}")
                            for t in range(2)
                        ]
                        for nt_ in range(NT):
                            t0 = nt_ * P
                            if t0 < 512:
                                chunks = [(t0, 512), (512, 1024)]
                            else:
                                chunks = [(t0, 1024)]
                            for cs, ce in chunks:
                                w = ce - cs
                                att_ps = ps_qk.tile(
                                    [P, 2, 512], f32, tag="qk", name="attps"
                                )
                                E = att_sb.tile([P, 2, 512], f32, tag="E", name="E")
                                for s in range(2):
                                    base = 64 * j + 32 * s
                                    nc.tensor.matmul(
                                        att_ps[:, s, :w],
                                        kT[base : base + 32, oc, ts(nt_, P)],
                                        qT[base : base + 32, oc, cs:ce],
                                        start=True,
                                        stop=True,
                                        tile_position=(96, 0) if base == 96 else None,
                                    )
                                nc.scalar.activation(
                                    E[:, :, :w], att_ps[:, :, :w], AF.Exp, scale=SCALE
                                )
                                if cs == t0:
                                    # diagonal block: keep t_local >= n_local
                                    for s in range(2):
                                        nc.gpsimd.affine_select(
                                            out=E[:, s, 0:P],
                                            in_=E[:, s, 0:P],
                                            compare_op=ALU.is_ge,
                                            fill=0.0,
                                            base=0,
                                            pattern=[[1, P]],
                                            channel_multiplier=-1,
                                        )
                                tcv = cs // 512
                                off = cs - 512 * tcv
                                for s in range(2):
                                    nc.tensor.matmul(
                                        av_ps[tcv][:, s, off : off + w],
                                        vaug[:, nt_, h, :],
                                        E[:, s, :w],
                                        start=(nt_ == 0),
                                        stop=(nt_ == (3 if tcv == 0 else 7)),
                                    )
                        for tcv in range(2):
                            nc.vector.tensor_copy(
                                P65[:, h, :, ts(tcv, 512)], av_ps[tcv][:, :, :]
                            )
                            for s in range(2):
                                nc.sync.dma_start(
                                    S_sb[2 * h + s : 2 * h + s + 1, ts(tcv, 512)],
                                    P65[64:65, h, s, ts(tcv, 512)],
                                )

                # ---- combine: out = P_pos/s_pos - lambda * P_neg/s_neg ----
                R = acc_sb.tile([16, T], f32)
                nc.vector.reciprocal(R, S_sb)
                for h in range(HPC):
                    for tcv in range(2):
                        # partition_broadcast needs its source on partition 0:
                        # stage the two reciprocal rows there via tiny DMAs.
                        R1h = att_sb.tile([1, 2, 512], f32, tag="R1h", bufs=2, name="R1h")
                        Rb = att_sb.tile([64, 2, 512], f32, tag="Rb", bufs=2, name="Rb")
                        for s in range(2):
                            nc.sync.dma_start(
                                R1h[:, s, :], R[2 * h + s : 2 * h + s + 1, ts(tcv, 512)]
                            )
                            nc.gpsimd.partition_broadcast(Rb[:, s, :], R1h[:, s, :])
                        m1 = att_sb.tile([64, 512], f32, tag="m1", bufs=2, name="m1")
                        m2 = att_sb.tile([64, 512], f32, tag="m2", bufs=2, name="m2")
                        nc.vector.tensor_mul(
                            m1, P65[0:64, h, 0, ts(tcv, 512)], Rb[:, 0, :]
                        )
                        nc.vector.tensor_mul(
                            m2, P65[0:64, h, 1, ts(tcv, 512)], Rb[:, 1, :]
                        )
                        o_sb = att_sb.tile([64, 512], f32, tag="osb", bufs=2, name="osb")
                        nc.vector.scalar_tensor_tensor(
                            o_sb,
                            in0=m2,
                            scalar=lam_neg_b,
                            in1=m1,
                            op0=ALU.mult,
                            op1=ALU.add,
                        )
                        nc.sync.dma_start(
                            outT_d[64 * h : 64 * h + 64, ts(tcv, 512)], o_sb
                        )

    nc.compile()
    return nc


def _get_state():
    if "nc" not in _STATE:
        from concourse.bass_utils import run_bass_kernel_spmd

        _STATE["nc"] = _build_nc()
        _STATE["run"] = run_bass_kernel_spmd
    return _STATE


def kernel(**inputs):
    st = _get_state()

    def f32c(a):
        return np.ascontiguousarray(np.asarray(a, dtype=np.float32))

    x = f32c(inputs["x"])
    ef = f32c(inputs["encoder_feature"])
    Wq, bq = f32c(inputs["Wq"]), f32c(inputs["bq"])
    Wk, bk = f32c(inputs["Wk"]), f32c(inputs["bk"])
    Wv, bv = f32c(inputs["Wv"]), f32c(inputs["bv"])
    lq1 = f32c(inputs["lambda_q1"]).reshape(1, HALF)
    lq2 = f32c(inputs["lambda_q2"]).reshape(1, HALF)
    lk1 = f32c(inputs["lambda_k1"]).reshape(1, HALF)
    lk2 = f32c(inputs["lambda_k2"]).reshape(1, HALF)

    in_maps = []
    for c in range(NCORES):
        b, hg = c // 2, c % 2
        sl = slice(hg * O, (hg + 1) * O)
        in_maps.append(
            {
                "x": f32c(x[b]),
                "ef": f32c(ef[b]),
                "wq": f32c(Wq[sl]),
                "wk": f32c(Wk[sl]),
                "wv": f32c(Wv[sl]),
                "bq": f32c(bq[sl]).reshape(1, O),
                "bk": f32c(bk[sl]).reshape(1, O),
                "bv": f32c(bv[sl]).reshape(1, O),
                "lq1": lq1,
                "lq2": lq2,
                "lk1": lk1,
                "lk2": lk2,
            }
        )

    res = st["run"](st["nc"], in_maps, core_ids=list(range(NCORES)))
    _STATE["last_results"] = res

    out = np.empty((B, T, HIDDEN), dtype=np.float32)
    for c in range(NCORES):
        b, hg = c // 2, c % 2
        out[b, :, hg * O : (hg + 1) * O] = res.results[c]["outT"].T
    return out
